# revision 19
# baseline (speedup 1.0000x reference)
"""Distributed Trainium2 kernel for nn_ComplexSVDTransform.

y = (x @ w) @ w_inv with w, w_inv 256x256 complex matrices derived from
tiny params (DFT, clamped perturbation, Neumann-series inverse). The two
matmuls are algebraically fused on the host into M = w @ w_inv (the
dim x dim matrices are tiny and replicated, per the sharding hint); the
device streams x through the matmul, data-parallel over the batch/token
rows across 8 NeuronCores.

M = I + dft @ P^9 @ dft^-1 with ||P|| <= 0.5 (Neumann telescoping), so
Im(M) is bounded by ||P||^9 — when that bound is below float32 output
precision (the case for any realistic alpha_logit), the imaginary output
plane is identically zero at output precision and only Re(M) is computed
on device ("re" mode, half the output traffic). Otherwise a full complex
kernel runs ("full" mode: Re/Im of M pre-interleaved into W[256,512] so
one N=512 matmul per k-chunk writes the complex64-interleaved layout
directly).
"""

from contextlib import ExitStack

import numpy as np

DIM = 256
NEUMANN_TERMS = 8
RHO_MAX = 0.5
N_CORES = 8
B, T = 4, 16384
ROWS = B * T  # 65536
R_CORE = ROWS // N_CORES  # 8192
P = 128
NB = R_CORE // P  # 64 row-blocks per core
G = 8  # rows per partition per DMA batch (contiguous per-partition chunks)
IM_NEGLIGIBLE = 1e-7  # vs unit-scale M; f32 ulp of 1.0 is 6e-8


def _fused_matrix(w_re, w_im, alpha_logit):
    """Host-side mirror of reference._get_matrix, fused: M = w @ w_inv.

    Computed in float64/complex128. M is insensitive to small numeric
    differences vs the f32 reference path: (I+P) @ sum_k(-P)^k
    telescopes to I + P^9 with ||P|| <= 0.5, so M ~= I regardless.
    """
    n = np.arange(DIM, dtype=np.float64)
    dft = np.exp(-2j * np.pi * np.outer(n, n) / DIM) / np.sqrt(DIM)
    dft_inv = dft.conj().T
    eye = np.eye(DIM, dtype=np.complex128)

    alpha = 1.0 / (1.0 + np.exp(-np.float64(alpha_logit)))
    beta = 1.0 - alpha
    alpha_scale = 1.0 + alpha * 0.001

    raw = (np.tanh(w_re.astype(np.float64)) + 1j * np.tanh(w_im.astype(np.float64))) * (
        DIM**-0.5
    )
    norm = np.linalg.norm(raw, ord=2)
    safe_norm = max(norm, float(np.finfo(np.float32).eps))
    scale = min(RHO_MAX / safe_norm, 1.0)
    perturb = (beta * scale) * raw

    learned = eye + perturb
    learned_inv = eye.copy()
    term = eye.copy()
    for _ in range(NEUMANN_TERMS):
        term = -(term @ perturb)
        learned_inv = learned_inv + term

    w = alpha_scale * (dft @ learned)
    w_inv = (learned_inv @ dft_inv) / alpha_scale
    return w @ w_inv


_CACHE = {}


def build_nc(mode):
    """Build + compile the per-core Bass kernel (shard shapes).

    mode "re": W is [256, 256] (= Re(M)), output [R_CORE, 256] f32.
    mode "full": W is [256, 512] (Re/Im interleaved), output
    [R_CORE, 512] f32 = complex64-interleaved rows.
    """
    if mode in _CACHE:
        return _CACHE[mode]

    import concourse.bass as bass  # noqa: F401
    import concourse.tile as tile
    from concourse import bacc, mybir

    f32 = mybir.dt.float32
    f32r = mybir.dt.float32r
    OD = DIM if mode == "re" else 2 * DIM  # output row width (f32 elems)

    nc = bacc.Bacc("TRN2", target_bir_lowering=False, debug=False, num_devices=N_CORES)
    x_d = nc.dram_tensor("x", [R_CORE, DIM], f32, kind="ExternalInput").ap()
    w_d = nc.dram_tensor("w", [DIM, OD], f32r, kind="ExternalInput").ap()
    id_d = nc.dram_tensor("ident", [P, P], f32, kind="ExternalInput").ap()
    out_d = nc.dram_tensor("out", [R_CORE, OD], f32, kind="ExternalOutput").ap()

    x_g = x_d.rearrange("(g p n) d -> g p (n d)", p=P, n=G)
    out_g = out_d.rearrange("(g p n) d -> g p (n d)", p=P, n=G)

    with tile.TileContext(nc) as tc, ExitStack() as ctx:
        const_pool = ctx.enter_context(tc.tile_pool(name="const", bufs=1))
        in_pool = ctx.enter_context(tc.tile_pool(name="xin", bufs=4))
        xt_pool = ctx.enter_context(tc.tile_pool(name="xt", bufs=6))
        out_pool = ctx.enter_context(tc.tile_pool(name="yout", bufs=3))
        psum_t = ctx.enter_context(tc.tile_pool(name="psum_t", bufs=3, space="PSUM"))
        psum_y = ctx.enter_context(tc.tile_pool(name="psum_y", bufs=4, space="PSUM"))

        ident = const_pool.tile([P, P], f32)
        nc.sync.dma_start(ident[:], id_d)
        w_sb = const_pool.tile([P, 2, OD], f32r)
        nc.sync.dma_start(w_sb[:], w_d.rearrange("(k p) n -> p k n", p=P))

        for g in range(NB // G):
            x_sb = in_pool.tile([P, G, DIM], f32)
            nc.sync.dma_start(x_sb[:], x_g[g])
            if mode == "re":
                y_sb = out_pool.tile([P, G, OD], f32)
            else:
                y_sb = out_pool.tile([P, G, DIM, 2], f32)
            for n in range(G):
                xT_ps = psum_t.tile([P, DIM], f32)
                nc.tensor.transpose(xT_ps[:, 0:P], x_sb[:, n, 0:P], ident[:])
                nc.tensor.transpose(xT_ps[:, P:DIM], x_sb[:, n, P:DIM], ident[:])
                # cast-copy into the f32r matmul operand: lossy (~13 bits) but
                # it only feeds the tiny correction matmul, never the x term
                xT_sb = xt_pool.tile([P, DIM], f32r)
                nc.scalar.copy(xT_sb[:], xT_ps[:])

                y_ps = psum_y.tile([P, OD], f32)
                nc.tensor.matmul(
                    y_ps[:], lhsT=xT_sb[:, 0:P], rhs=w_sb[:, 0], start=True, stop=False
                )
                nc.tensor.matmul(
                    y_ps[:], lhsT=xT_sb[:, P:DIM], rhs=w_sb[:, 1], start=False, stop=True
                )
                # y = x + x @ (M - I): the matmul carries only the small
                # correction term, so f32r truncation never touches the
                # dominant x term (exact f32 via the DVE add).
                if mode == "re":
                    nc.vector.tensor_add(y_sb[:, n], x_sb[:, n], y_ps[:])
                else:
                    y4 = y_ps[:].rearrange("p (d two) -> p d two", two=2)
                    nc.vector.tensor_add(y_sb[:, n, :, 0], x_sb[:, n], y4[:, :, 0])
                    nc.vector.tensor_copy(y_sb[:, n, :, 1], y4[:, :, 1])
            # outputs on the ACT HWDGE queue so they don't head-of-line
            # block input loads on the SP queue
            nc.scalar.dma_start(out_g[g], y_sb[:])

    nc.compile()
    _CACHE[mode] = nc
    return nc


def prepare(x, w_re, w_im, alpha_logit):
    """Returns (mode, in_maps)."""
    M = _fused_matrix(
        np.asarray(w_re), np.asarray(w_im), np.asarray(alpha_logit, dtype=np.float64)
    )
    A = M.real.astype(np.float32)
    Bm = M.imag.astype(np.float32)
    mode = "re" if float(np.abs(Bm).max()) < IM_NEGLIGIBLE else "full"
    E = A - np.eye(DIM, dtype=np.float32)
    if mode == "re":
        W = np.ascontiguousarray(E)
    else:
        W = np.empty((DIM, 2 * DIM), dtype=np.float32)
        W[:, 0::2] = E
        W[:, 1::2] = Bm
    ident = np.eye(P, dtype=np.float32)
    xf = np.ascontiguousarray(np.asarray(x, dtype=np.float32).reshape(ROWS, DIM))
    in_maps = [
        {"x": xf[c * R_CORE : (c + 1) * R_CORE], "w": W, "ident": ident}
        for c in range(N_CORES)
    ]
    return mode, in_maps


def assemble_output(mode, results):
    if mode == "re":
        out = np.zeros((ROWS, DIM), dtype=np.complex64)
        re = out.reshape(ROWS, DIM).view(np.float32).reshape(ROWS, 2 * DIM)
        for c in range(N_CORES):
            re[c * R_CORE : (c + 1) * R_CORE, 0::2] = results[c]["out"]
    else:
        out = np.empty((ROWS, DIM), dtype=np.complex64)
        for c in range(N_CORES):
            plane = np.ascontiguousarray(results[c]["out"])  # (R_CORE, 512) f32
            out[c * R_CORE : (c + 1) * R_CORE] = plane.view(np.complex64)
    return out.reshape(B, T, DIM)


def kernel(x, w_re, w_im, alpha_logit):
    from concourse import bass_utils

    mode, in_maps = prepare(x, w_re, w_im, alpha_logit)
    nc = build_nc(mode)
    res = bass_utils.run_bass_kernel_spmd(nc, in_maps, list(range(N_CORES)))
    return assemble_output(mode, res.results)
